# revision 25
# baseline (speedup 1.0000x reference)
"""Trainium2 Bass kernel for nn_Actor (GRU-over-vehicles + MLP head), v3.

Data parallel: B=16384 split across 8 cores (2048 rows each), params replicated.

v3 vs v2:
- GRU x-side drops the two bias-only MM chunks (b_hhn now folded into the
  DVE scalar_tensor_tensor u-op as an exact fp32 per-partition scalar):
  6 x-side MMs per (t,g) instead of 8.
- GRU elementwise rebalanced: d = h - n runs on GpSimd; u = (pHN+b)*r is a
  fused STT on DVE; no ScalarE xn evacuation.
- MLP runs in fp8 e4m3 residual (hi+lo) with DoubleRow perf mode: each
  logical matmul = Whi*Ahi + Whi*Alo + Wlo*Ahi, each product a DoubleRow MM
  pairing two K-tiles at 2 output columns/cycle. Weights are pre-split on
  host (scale 32); activations are split on device (scale 8, psum = 256x).
  Biases are exact fp32, applied in the DVE relu evacuation (tensor_scalar
  (psum + 256b) max 0), with hi-quantization on ScalarE and lo on DVE.
- MLP weight DMAs are interleaved into the GRU t-loop so xT prefetches are
  never queued behind them (fixes a ~13us tensor stall at startup).
"""

import numpy as np
import ml_dtypes

import concourse.bass as bass
import concourse.tile as tile
from concourse import bacc
from concourse import mybir
from concourse.bass_utils import run_bass_kernel_spmd

BF16 = mybir.dt.bfloat16
FP8 = mybir.dt.float8e4
F32 = mybir.dt.float32
Act = mybir.ActivationFunctionType
Alu = mybir.AluOpType
DR = mybir.MatmulPerfMode.DoubleRow

B, V, F, H = 16384, 20, 15, 256
NCORES = 8
BL = B // NCORES          # 2048 batch rows per core
GN = 512                  # batch-group width (PSUM bank = 512 fp32)
NG = BL // GN             # 4 groups

_NC_CACHE = {}
DBG = False


def _build_nc():
    nc = bacc.Bacc("TRN2", target_bir_lowering=False, debug=False)

    xT_d = nc.dram_tensor("xT", [V, 128, BL], BF16, kind="ExternalInput")
    wih_d = nc.dram_tensor("wih_all", [128, 768], BF16, kind="ExternalInput")
    whh_d = nc.dram_tensor("w_hhT", [2, 128, 768], BF16, kind="ExternalInput")
    w1x_d = nc.dram_tensor("w1T_x", [16, 1024], BF16, kind="ExternalInput")
    w1h_d = [nc.dram_tensor(f"w1h_{s}", [128, 2, 1024], FP8, kind="ExternalInput")
             for s in ("hi", "lo")]
    w2_d = [nc.dram_tensor(f"w2_{s}", [4, 128, 2, 1024], FP8, kind="ExternalInput")
            for s in ("hi", "lo")]
    w3_d = [nc.dram_tensor(f"w3_{s}", [4, 128, 2, 512], FP8, kind="ExternalInput")
            for s in ("hi", "lo")]
    w4_d = [nc.dram_tensor(f"w4_{s}", [2, 128, 2, 256], FP8, kind="ExternalInput")
            for s in ("hi", "lo")]
    wp_d = nc.dram_tensor("wpT", [2, 128, 1], BF16, kind="ExternalInput")
    bhn_d = nc.dram_tensor("bhh_n", [128, 2], F32, kind="ExternalInput")
    # MLP bias tables, 22 chunk columns (L1:0-7, L2:8-15, L3:16-19, L4:20-21)
    b8_d = nc.dram_tensor("b8a", [128, 22], F32, kind="ExternalInput")
    bn_d = nc.dram_tensor("bna", [128, 22], F32, kind="ExternalInput")
    bq_d = nc.dram_tensor("bqa", [128, 22], F32, kind="ExternalInput")
    bp_d = nc.dram_tensor("bp", [1, 1], F32, kind="ExternalInput")
    out_d = nc.dram_tensor("out", [1, BL], F32, kind="ExternalOutput")
    if DBG:
        hdbg_d = nc.dram_tensor("hdbg", [NG, 128, 1024], BF16,
                                kind="ExternalOutput")
        t1dbg_d = nc.dram_tensor("t1dbg", [128, 2048], BF16,
                                 kind="ExternalOutput")
        rzdbg_d = nc.dram_tensor("rzdbg", [2, 128, 2048], BF16,
                                 kind="ExternalOutput")
        ndbg_d = nc.dram_tensor("ndbg", [2, 128, 1024], BF16,
                                kind="ExternalOutput")
        udbg_d = nc.dram_tensor("udbg", [2, 128, 1024], BF16,
                                kind="ExternalOutput")

    with tile.TileContext(nc) as tc:
        with (
            tc.tile_pool(name="const", bufs=1) as consts,
            tc.tile_pool(name="psum", bufs=2, space=bass.MemorySpace.PSUM) as psum,
            tc.tile_pool(name="work", bufs=3) as work,
            tc.tile_pool(name="mlp", bufs=8) as mlp,
        ):
            def cload(dram_ap, shape, dtype, tag):
                t = consts.tile(shape, dtype, tag=tag, name=tag)
                nc.sync.dma_start(t[:], dram_ap)
                return t

            # ---- critical-path DMAs first: wih, x0, whh, early xT ----
            wih = cload(wih_d[:], [128, 768], BF16, "wih")
            x0 = consts.tile([128, BL], BF16, tag="x0", name="x0")
            for g0 in range(NG):  # per-group slices: first group lands first
                nc.sync.dma_start(x0[:, GN * g0 : GN * (g0 + 1)],
                                  xT_d[0][:, GN * g0 : GN * (g0 + 1)])
            whh = [cload(whh_d[i], [128, 768], BF16, f"whh{i}") for i in range(2)]
            bhn = cload(bhn_d[:], [128, 2], F32, "bhn")

            xtiles = {0: x0}

            def xtile(t):
                if t not in xtiles:
                    xt_ = work.tile([128, BL], BF16, tag="xt", name="xt", bufs=4)
                    nc.sync.dma_start(xt_[:], xT_d[t])
                    xtiles[t] = xt_
                return xtiles[t]

            xtile(1)
            xtile(2)
            xtile(3)

            # ---- MLP weight tiles: allocate now, DMA inside the t-loop ----
            w1x = consts.tile([16, 1024], BF16, tag="w1x", name="w1x")
            w1h = [consts.tile([128, 2, 1024], FP8, tag=f"w1h{s}", name=f"w1h{s}")
                   for s in range(2)]
            w2 = [[consts.tile([128, 2, 1024], FP8, tag=f"w2{s}_{k}",
                               name=f"w2{s}_{k}") for k in range(4)]
                  for s in range(2)]
            w3 = [[consts.tile([128, 2, 512], FP8, tag=f"w3{s}_{k}",
                               name=f"w3{s}_{k}") for k in range(4)]
                  for s in range(2)]
            w4 = [[consts.tile([128, 2, 256], FP8, tag=f"w4{s}_{k}",
                               name=f"w4{s}_{k}") for k in range(2)]
                  for s in range(2)]
            wp = [consts.tile([128, 1], BF16, tag=f"wp{s}", name=f"wp{s}")
                  for s in range(2)]
            b8a = consts.tile([128, 22], F32, tag="b8a", name="b8a")
            bna = consts.tile([128, 22], F32, tag="bna", name="bna")
            bqa = consts.tile([128, 22], F32, tag="bqa", name="bqa")
            bp = consts.tile([1, 1], F32, tag="bp", name="bp")

            deferred = [
                (bp, bp_d[:]), (b8a, b8_d[:]), (bna, bn_d[:]),
                (bqa, bq_d[:]),
                (w1x, w1x_d[:]),
                (w1h[0], w1h_d[0][:]), (w1h[1], w1h_d[1][:]),
            ]
            for k in range(4):
                deferred += [(w2[0][k], w2_d[0][k]), (w2[1][k], w2_d[1][k])]
            for k in range(4):
                deferred += [(w3[0][k], w3_d[0][k]), (w3[1][k], w3_d[1][k])]
            for k in range(2):
                deferred += [(w4[0][k], w4_d[0][k]), (w4[1][k], w4_d[1][k])]
            deferred += [(wp[0], wp_d[0]), (wp[1], wp_d[1])]

            def pump_deferred(nmax):
                for _ in range(nmax):
                    if deferred:
                        t_, ap_ = deferred.pop(0)
                        nc.sync.dma_start(t_[:], ap_)

            oT = consts.tile([1, BL], F32, tag="oT", name="oT")

            mm = nc.tensor.matmul
            h_cur = [None] * NG
            pending = [None]  # (t, g, rz, n, d) awaiting h-update

            def flush_tail():
                if pending[0] is None:
                    return
                tt, gg, rz_p, n_p, d_p = pending[0]
                pending[0] = None
                h_new = work.tile([128, 1024], BF16, tag=f"h{gg}",
                                  name=f"h{gg}", bufs=2)
                a_ = work.tile([128, 1024], BF16, tag="a", name="a", bufs=2)
                if tt == 0:
                    nc.vector.tensor_mul(a_[:], rz_p[:, 1024:2048], n_p[:])
                    nc.vector.tensor_sub(h_new[:], n_p[:], a_[:])
                else:
                    nc.vector.tensor_mul(a_[:], rz_p[:, 1024:2048], d_p[:])
                    nc.vector.tensor_add(h_new[:], n_p[:], a_[:])
                h_cur[gg] = h_new

            # ---------------- GRU over V=20 vehicle steps ----------------
            for t in range(V):
                xa = xtiles[t] if t in xtiles else xtile(t)
                xtile(min(t + 3, V - 1))  # prefetch
                if t >= 1:
                    pump_deferred(3)
                for g in range(NG):
                    gs = slice(GN * g, GN * (g + 1))

                    pR = psum.tile([128, 1024], F32, tag="pR", name="pR", bufs=1)
                    pZ = psum.tile([128, 1024], F32, tag="pZ", name="pZ", bufs=1)
                    pXN = psum.tile([128, 1024], F32, tag="pXN", name="pXN", bufs=1)
                    pHN = None
                    if t > 0:
                        pHN = psum.tile([128, 1024], F32, tag="pHN", name="pHN",
                                        bufs=1)

                    # x-side: 6 K=16 matmuls row-tiled to strips 0/32/64/96.
                    # r/z biases (b_ih+b_hh) and b_ihn ride the ones-row.
                    rz_dst = [pR[:, 0:512], pR[:, 512:1024],
                              pZ[:, 0:512], pZ[:, 512:1024]]
                    for c in range(4):  # r0 r1 z0 z1
                        s = 32 * c
                        mm(rz_dst[c],
                           wih[s : s + 16, 128 * c : 128 * (c + 1)],
                           xa[s : s + 16, gs],
                           start=True, stop=(t == 0),
                           tile_position=(s, 0))
                    for m in range(2):  # xn m0, xn m1 (incl b_ihn)
                        s = 32 * m
                        mm(pXN[:, 512 * m : 512 * (m + 1)],
                           wih[s : s + 16, 128 * (4 + m) : 128 * (5 + m)],
                           xa[s : s + 16, gs],
                           start=True, stop=True,
                           tile_position=(s, 0))

                    # h-side (t>0)
                    if t > 0:
                        hg = h_cur[g]
                        for c in range(4):
                            for k in range(2):
                                mm(rz_dst[c],
                                   whh[k][:, 128 * c : 128 * (c + 1)],
                                   hg[:, 512 * k : 512 * (k + 1)],
                                   start=False, stop=(k == 1))
                        for m in range(2):
                            for k in range(2):
                                mm(pHN[:, 512 * m : 512 * (m + 1)],
                                   whh[k][:, 512 + 128 * m : 512 + 128 * (m + 1)],
                                   hg[:, 512 * k : 512 * (k + 1)],
                                   start=(k == 0), stop=(k == 1))

                    rz = work.tile([128, 2048], BF16, tag="rz", name="rz", bufs=3)
                    nc.scalar.activation(rz[:, 0:1024], pR[:], Act.Sigmoid)
                    nc.scalar.activation(rz[:, 1024:2048], pZ[:], Act.Sigmoid)

                    # flush previous group's h-update before this group's u/w
                    flush_tail()

                    u_ = work.tile([128, 1024], BF16, tag="u", name="u", bufs=2)
                    if t == 0:
                        # pHN = 0: u = r * b_hhn
                        for m in range(2):
                            nc.vector.tensor_scalar_mul(
                                u_[:, 512 * m : 512 * (m + 1)],
                                rz[:, 512 * m : 512 * (m + 1)],
                                bhn[:, m : m + 1])
                    else:
                        # u = (pHN + b_hhn) * r, fused STT per m-half
                        for m in range(2):
                            nc.vector.scalar_tensor_tensor(
                                u_[:, 512 * m : 512 * (m + 1)],
                                pHN[:, 512 * m : 512 * (m + 1)],
                                bhn[:, m : m + 1],
                                rz[:, 512 * m : 512 * (m + 1)],
                                op0=Alu.add, op1=Alu.mult)
                    w_ = work.tile([128, 1024], BF16, tag="w", name="w", bufs=2)
                    nc.vector.tensor_add(w_[:], u_[:], pXN[:])
                    n_ = work.tile([128, 1024], BF16, tag="n", name="n", bufs=2)
                    nc.scalar.activation(n_[:], w_[:], Act.Tanh)

                    d_ = None
                    if t > 0:
                        d_ = work.tile([128, 1024], BF16, tag="d", name="d",
                                       bufs=2)
                        nc.gpsimd.tensor_sub(d_[:], h_cur[g][:], n_[:])

                    if DBG and g == 0 and t < 2:
                        nc.sync.dma_start(rzdbg_d[t], rz[:])
                        nc.sync.dma_start(ndbg_d[t], n_[:])
                        nc.sync.dma_start(udbg_d[t], u_[:])

                    pending[0] = (t, g, rz, n_, d_)

            flush_tail()
            pump_deferred(99)

            if DBG:
                for g in range(NG):
                    nc.sync.dma_start(hdbg_d[g], h_cur[g][:])

            # ---------------- split final h into fp8 hi/lo (x8) ----------
            hfh = mlp.tile([128, 2, BL], FP8, tag="hfh", name="hfh", bufs=1)
            hfl = mlp.tile([128, 2, BL], FP8, tag="hfl", name="hfl", bufs=1)
            for g in range(NG):
                gs = slice(GN * g, GN * (g + 1))
                src = h_cur[g][:].rearrange("p (k n) -> p k n", k=2)
                nc.vector.tensor_scalar_mul(hfh[:, :, gs], src, 8.0)
                nc.vector.scalar_tensor_tensor(
                    hfl[:, :, gs], src, 8.0, hfh[:, :, gs],
                    op0=Alu.mult, op1=Alu.subtract)

            # ---------------- MLP head (fp8 residual, DoubleRow) ---------
            GSL = [slice(GN * g, GN * (g + 1)) for g in range(NG)]

            def evac(m_, pq0, pq1, co, out_hi, out_lo, kt, scalar_relu=False):
                """psum (256x) -> relu -> fp8 hi/lo (8x) into pair slot kt.

                GpSimd cannot touch PSUM and has no STT, so:
                half 0: ScalarE ACT Relu(psum/32 + 8b) -> t_=8a; ScalarE
                        Copy -> hi; GpSimd tensor_sub -> lo.
                half 1: DVE tensor_scalar (psum max -256b) add 256b ->
                        t_=256a; ScalarE Copy/32 -> hi; DVE STT -> lo.
                With scalar_relu=True (L4, whose t_ feeds the bf16 head MM),
                both halves take the ScalarE path so t_ is 8a throughout.
                """
                t_ = mlp.tile([128, 2048], BF16, tag="ts", name="ts", bufs=3)
                cs0, cs1 = slice(0, 1024), slice(1024, 2048)
                cb = slice(co, co + 1)
                nc.scalar.activation(t_[:, cs0], pq0[:], Act.Relu,
                                     bias=b8a[:, cb], scale=1.0 / 32.0)
                nc.scalar.activation(out_hi[:, kt, cs0], t_[:, cs0], Act.Copy)
                nc.gpsimd.tensor_sub(out_lo[:, kt, cs0], t_[:, cs0],
                                     out_hi[:, kt, cs0])
                if scalar_relu:
                    nc.scalar.activation(t_[:, cs1], pq1[:], Act.Relu,
                                         bias=b8a[:, cb], scale=1.0 / 32.0)
                    nc.scalar.activation(out_hi[:, kt, cs1], t_[:, cs1],
                                         Act.Copy)
                    nc.vector.tensor_sub(out_lo[:, kt, cs1], t_[:, cs1],
                                         out_hi[:, kt, cs1])
                else:
                    nc.vector.tensor_scalar(t_[:, cs1], pq1[:], bna[:, cb],
                                            bqa[:, cb], op0=Alu.max,
                                            op1=Alu.add)
                    nc.scalar.activation(out_hi[:, kt, cs1], t_[:, cs1],
                                         Act.Copy, scale=1.0 / 32.0)
                    nc.vector.scalar_tensor_tensor(
                        out_lo[:, kt, cs1], t_[:, cs1], 1.0 / 32.0,
                        out_hi[:, kt, cs1], op0=Alu.mult, op1=Alu.subtract)
                return t_

            def pair_tiles(tagbase, n_pairs, width=2048):
                his = [mlp.tile([128, 2, width], FP8, tag=f"{tagbase}h{i}",
                                name=f"{tagbase}h{i}", bufs=1)
                       for i in range(n_pairs)]
                los = [mlp.tile([128, 2, width], FP8, tag=f"{tagbase}l{i}",
                                name=f"{tagbase}l{i}", bufs=1)
                       for i in range(n_pairs)]
                return his, los

            tagp = [("pR", "pZ"), ("pXN", "pHN")]

            def mlp_layer(n_out_chunks, n_kpairs, whi_of, wlo_of, rhs_hi_of,
                          rhs_lo_of, chunk_off, out_hi, out_lo, extra_mm=None,
                          scalar_relu=False):
                stages = []
                for m_ in range(n_out_chunks):
                    tg = tagp[m_ % 2]
                    pq0 = psum.tile([128, 1024], F32, tag=tg[0], name="pq0",
                                    bufs=1)
                    pq1 = psum.tile([128, 1024], F32, tag=tg[1], name="pq1",
                                    bufs=1)
                    halves = [pq0[:, 0:512], pq0[:, 512:1024],
                              pq1[:, 0:512], pq1[:, 512:1024]]
                    n_mm = n_kpairs * 3 + (1 if extra_mm else 0)
                    for g in range(NG):
                        i = 0
                        if extra_mm:
                            extra_mm(halves[g], m_, g, i == 0, i == n_mm - 1)
                            i += 1
                        for kp in range(n_kpairs):
                            for lhs, rhs in (
                                (whi_of(kp, m_), rhs_hi_of(kp, g)),
                                (whi_of(kp, m_), rhs_lo_of(kp, g)),
                                (wlo_of(kp, m_), rhs_hi_of(kp, g)),
                            ):
                                mm(halves[g], lhs, rhs,
                                   start=(i == 0), stop=(i == n_mm - 1),
                                   perf_mode=DR)
                                i += 1
                    stages.append(evac(m_, pq0, pq1, chunk_off + m_,
                                       out_hi[m_ // 2], out_lo[m_ // 2],
                                       m_ % 2, scalar_relu=scalar_relu))
                return stages

            a1h, a1l = pair_tiles("pa", 4)

            def l1_extra(dst, m_, g, start, stop):
                mm(dst, w1x[0:16, 128 * m_ : 128 * (m_ + 1)],
                   x0[0:16, GSL[g]], start=start, stop=stop,
                   tile_position=(0, 0))

            t1s = mlp_layer(
                8, 1,
                lambda kp, m_: w1h[0][:, :, 128 * m_ : 128 * (m_ + 1)],
                lambda kp, m_: w1h[1][:, :, 128 * m_ : 128 * (m_ + 1)],
                lambda kp, g: hfh[:, :, GSL[g]],
                lambda kp, g: hfl[:, :, GSL[g]],
                0,
                a1h, a1l, extra_mm=l1_extra)
            if DBG:
                nc.sync.dma_start(t1dbg_d[:], t1s[0][:])

            a2h, a2l = pair_tiles("pb", 4)
            mlp_layer(
                8, 4,
                lambda kp, m_: w2[0][kp][:, :, 128 * m_ : 128 * (m_ + 1)],
                lambda kp, m_: w2[1][kp][:, :, 128 * m_ : 128 * (m_ + 1)],
                lambda kp, g: a1h[kp][:, :, GSL[g]],
                lambda kp, g: a1l[kp][:, :, GSL[g]],
                8,
                a2h, a2l)

            # reuse L1's pair buffers (generation 2) — a1 is dead after L2
            a3h, a3l = pair_tiles("pa", 2)
            mlp_layer(
                4, 4,
                lambda kp, m_: w3[0][kp][:, :, 128 * m_ : 128 * (m_ + 1)],
                lambda kp, m_: w3[1][kp][:, :, 128 * m_ : 128 * (m_ + 1)],
                lambda kp, g: a2h[kp][:, :, GSL[g]],
                lambda kp, g: a2l[kp][:, :, GSL[g]],
                16,
                a3h, a3l)

            # reuse L2's pair buffers (generation 2) — a2 is dead after L3
            a4h, a4l = pair_tiles("pb", 1)
            t4 = mlp_layer(
                2, 2,
                lambda kp, m_: w4[0][kp][:, :, 128 * m_ : 128 * (m_ + 1)],
                lambda kp, m_: w4[1][kp][:, :, 128 * m_ : 128 * (m_ + 1)],
                lambda kp, g: a3h[kp][:, :, GSL[g]],
                lambda kp, g: a3l[kp][:, :, GSL[g]],
                20,
                a4h, a4l, scalar_relu=True)

            # head: [1, BL] = tanh(a4 . Wp / 256 + bp), bf16 on L4's
            # 8x-scaled staging tiles (wp pre-scaled x32 on host)
            po0 = psum.tile([1, 1024], F32, tag="pR", name="po0", bufs=1)
            po1 = psum.tile([1, 1024], F32, tag="pZ", name="po1", bufs=1)
            phalf = [po0[0:1, 0:512], po0[0:1, 512:1024],
                     po1[0:1, 0:512], po1[0:1, 512:1024]]
            for g in range(NG):
                for k in range(2):
                    mm(phalf[g], wp[k][:, 0:1], t4[k][:, GSL[g]],
                       start=(k == 0), stop=(k == 1))
            nc.scalar.activation(oT[0:1, 0:1024], po0[0:1, :], Act.Tanh,
                                 bias=bp[0:1, 0:1], scale=1.0 / 256.0)
            nc.scalar.activation(oT[0:1, 1024:2048], po1[0:1, :], Act.Tanh,
                                 bias=bp[0:1, 0:1], scale=1.0 / 256.0)

            nc.sync.dma_start(out_d[:], oT[:])

    nc.compile()
    return nc


def _get_nc():
    if "nc" not in _NC_CACHE:
        _NC_CACHE["nc"] = _build_nc()
    return _NC_CACHE["nc"]


def _split8(x):
    hi = x.astype(ml_dtypes.float8_e4m3)
    lo = (x - hi.astype(np.float32)).astype(ml_dtypes.float8_e4m3)
    return hi, lo


def _prep_shared(inputs):
    f4 = np.float32
    bf = ml_dtypes.bfloat16

    def g(name):
        return np.asarray(inputs[name], dtype=f4)

    W_ih, W_hh = g("W_ih"), g("W_hh")
    b_ih, b_hh = g("b_ih"), g("b_hh")
    W1, W2, W3, W4, Wp = g("W1"), g("W2"), g("W3"), g("W4"), g("Wp")
    b1, b2, b3, b4, bp = g("b1"), g("b2"), g("b3"), g("b4"), g("bp")

    # wih_all [128, 768]: 6 column blocks; block c sits at partition strip
    # 32*(c%4). blocks 0-3: r/z gate chunks (ones-row bias = b_ih+b_hh);
    # blocks 4-5: xn chunks (ones-row bias = b_ih n-part).
    wih_all = np.zeros((128, 768), dtype=f4)
    bsum = b_ih + b_hh
    for c in range(4):
        s = 32 * c
        wih_all[s : s + 15, 128 * c : 128 * (c + 1)] = W_ih[128 * c : 128 * (c + 1)].T
        wih_all[s + 15, 128 * c : 128 * (c + 1)] = bsum[128 * c : 128 * (c + 1)]
    for m in range(2):
        c, s = 4 + m, 32 * m
        lo = 512 + 128 * m
        wih_all[s : s + 15, 128 * c : 128 * (c + 1)] = W_ih[lo : lo + 128].T
        wih_all[s + 15, 128 * c : 128 * (c + 1)] = b_ih[lo : lo + 128]

    w1x = np.zeros((16, 1024), dtype=f4)
    w1x[0:15] = 256.0 * W1.T[0:15]

    def pairw(WT, n_kp, m_):
        # [K, M] fp32*32 -> hi/lo [n_kp, 128, 2, M] fp8
        arr = (32.0 * WT).reshape(n_kp, 2, 128, m_).transpose(0, 2, 1, 3)
        return _split8(np.ascontiguousarray(arr))

    w1h_hi, w1h_lo = pairw(W1.T[15:271], 1, 1024)
    w2_hi, w2_lo = pairw(W2.T, 4, 1024)
    w3_hi, w3_lo = pairw(W3.T, 4, 512)
    w4_hi, w4_lo = pairw(W4.T, 2, 256)

    # bias table [128, 22]: chunk c of layer l in column; per-partition rows
    ball = np.zeros((128, 22), dtype=f4)
    for off, bv in ((0, b1), (8, b2), (16, b3), (20, b4)):
        nch = bv.shape[0] // 128
        ball[:, off : off + nch] = bv.reshape(nch, 128).T

    shared = {
        "wih_all": wih_all.astype(bf),
        "w_hhT": np.ascontiguousarray(W_hh.T.reshape(2, 128, 768)).astype(bf),
        "w1T_x": w1x.astype(bf),
        "w1h_hi": w1h_hi[0], "w1h_lo": w1h_lo[0],
        "w2_hi": w2_hi, "w2_lo": w2_lo,
        "w3_hi": w3_hi, "w3_lo": w3_lo,
        "w4_hi": w4_hi, "w4_lo": w4_lo,
        "wpT": np.ascontiguousarray((32.0 * Wp.T).reshape(2, 128, 1)).astype(bf),
        "bhh_n": np.ascontiguousarray(b_hh[512:768].reshape(2, 128).T),
        "b8a": 8.0 * ball, "bna": -256.0 * ball, "bqa": 256.0 * ball,
        "bp": bp.reshape(1, 1).astype(f4),
    }
    return shared


def _prep_xT(state_core):
    """state [BL, 20, 15] f32 -> [20, 128, BL] bf16: features replicated at
    partition strips 0/32/64/96; row 15 of each strip is the all-ones bias
    row; rows 16-31 zero."""
    s = state_core.transpose(1, 2, 0)  # [20, 15, BL]
    xp = np.zeros((V, 4, 32, BL), dtype=np.float32)
    xp[:, :, :F, :] = s[:, None, :, :]
    xp[:, :, 15, :] = 1.0
    return xp.reshape(V, 128, BL).astype(ml_dtypes.bfloat16)


def run(inputs, trace=False):
    nc = _get_nc()
    shared = _prep_shared(inputs)
    state = np.asarray(inputs["state"], dtype=np.float32)
    in_maps = []
    for c in range(NCORES):
        m = dict(shared)
        m["xT"] = _prep_xT(state[BL * c : BL * (c + 1)])
        in_maps.append(m)
    res = run_bass_kernel_spmd(nc, in_maps, list(range(NCORES)), trace=trace)
    out = np.concatenate(
        [np.asarray(res.results[c]["out"]).reshape(BL) for c in range(NCORES)]
    )
    return out.reshape(B, 1).astype(np.float32), res


def kernel(**inputs):
    out, _ = run(inputs, trace=False)
    return out


# revision 26
# speedup vs baseline: 1.0174x; 1.0174x over previous
"""Trainium2 Bass kernel for nn_Actor (GRU-over-vehicles + MLP head), v4.

Data parallel: B=16384 split across 8 cores (2048 rows each), params replicated.

All bf16 matmuls (fp8 DoubleRow measured at 1 col/cycle on HW, so the
hi+lo residual path is net slower than bf16 — reverted).

v4 vs v2 (the 485us baseline):
- GRU x-side drops the two bias-only MM chunks: 6 x-side MMs per (t,g)
  instead of 8 (b_hhn folded into a fused DVE scalar_tensor_tensor
  u = (pHN + b) * r with an exact fp32 per-partition scalar).
- GRU MM issue order per group: [rz h-side + x-side interleaved per chunk]
  -> [hn h-side] -> [xn x-side]. pR/pZ writers lead, so the next group's
  sigmoid-drain WAR resolves before the tensor engine reaches them, and
  pHN/pXN writers trail, giving the previous group's DVE reads time.
- h-update is software-pipelined TWO groups deep (d = h - n runs on GpSimd,
  whose ~2.4us latency would stall a 1-deep pipeline).
- No ScalarE xn evacuation; DVE reads pXN directly.
- MLP weight DMAs are interleaved into the GRU t-loop so xT prefetches are
  never queued behind them (fixes a ~13us tensor stall at startup).
"""

import numpy as np
import ml_dtypes

import concourse.bass as bass
import concourse.tile as tile
from concourse import bacc
from concourse import mybir
from concourse.bass_utils import run_bass_kernel_spmd

BF16 = mybir.dt.bfloat16
F32 = mybir.dt.float32
Act = mybir.ActivationFunctionType
Alu = mybir.AluOpType

B, V, F, H = 16384, 20, 15, 256
NCORES = 8
BL = B // NCORES          # 2048 batch rows per core
GN = 512                  # batch-group width (PSUM bank = 512 fp32)
NG = BL // GN             # 4 groups

_NC_CACHE = {}


def _build_nc():
    nc = bacc.Bacc("TRN2", target_bir_lowering=False, debug=False)

    xT_d = nc.dram_tensor("xT", [V, 128, BL], BF16, kind="ExternalInput")
    wih_d = nc.dram_tensor("wih_all", [128, 768], BF16, kind="ExternalInput")
    whh_d = nc.dram_tensor("w_hhT", [2, 128, 768], BF16, kind="ExternalInput")
    w1x_d = nc.dram_tensor("w1T_x", [16, 1024], BF16, kind="ExternalInput")
    w1h_d = nc.dram_tensor("w1T_h", [2, 128, 1024], BF16, kind="ExternalInput")
    w2_d = nc.dram_tensor("w2T", [8, 128, 1024], BF16, kind="ExternalInput")
    w3_d = nc.dram_tensor("w3T", [8, 128, 512], BF16, kind="ExternalInput")
    w4_d = nc.dram_tensor("w4T", [4, 128, 256], BF16, kind="ExternalInput")
    wp_d = nc.dram_tensor("wpT", [2, 128, 1], BF16, kind="ExternalInput")
    bhn_d = nc.dram_tensor("bhh_n", [128, 2], F32, kind="ExternalInput")
    b2_d = nc.dram_tensor("b2", [128, 8], F32, kind="ExternalInput")
    b3_d = nc.dram_tensor("b3", [128, 4], F32, kind="ExternalInput")
    b4_d = nc.dram_tensor("b4", [128, 2], F32, kind="ExternalInput")
    bp_d = nc.dram_tensor("bp", [1, 1], F32, kind="ExternalInput")
    out_d = nc.dram_tensor("out", [1, BL], F32, kind="ExternalOutput")

    with tile.TileContext(nc) as tc:
        with (
            tc.tile_pool(name="const", bufs=1) as consts,
            tc.tile_pool(name="psum", bufs=2, space=bass.MemorySpace.PSUM) as psum,
            tc.tile_pool(name="work", bufs=3) as work,
            tc.tile_pool(name="mlp", bufs=8) as mlp,
        ):
            def cload(dram_ap, shape, dtype, tag):
                t = consts.tile(shape, dtype, tag=tag, name=tag)
                nc.sync.dma_start(t[:], dram_ap)
                return t

            # ---- critical-path DMAs first: wih, x0, whh, early xT ----
            wih = cload(wih_d[:], [128, 768], BF16, "wih")
            x0 = consts.tile([128, BL], BF16, tag="x0", name="x0")
            for g0 in range(NG):  # per-group slices: first group lands first
                nc.sync.dma_start(x0[:, GN * g0 : GN * (g0 + 1)],
                                  xT_d[0][:, GN * g0 : GN * (g0 + 1)])
            whh = [cload(whh_d[i], [128, 768], BF16, f"whh{i}") for i in range(2)]
            bhn = cload(bhn_d[:], [128, 2], F32, "bhn")

            xtiles = {0: x0}

            def xtile(t):
                if t not in xtiles:
                    xt_ = work.tile([128, BL], BF16, tag="xt", name="xt", bufs=3)
                    nc.sync.dma_start(xt_[:], xT_d[t])
                    xtiles[t] = xt_
                return xtiles[t]

            xtile(1)
            xtile(2)

            # ---- MLP weight tiles: allocate now, DMA inside the t-loop ----
            w1x = consts.tile([16, 1024], BF16, tag="w1x", name="w1x")
            w1h = [consts.tile([128, 1024], BF16, tag=f"w1h{i}", name=f"w1h{i}")
                   for i in range(2)]
            w2 = [consts.tile([128, 1024], BF16, tag=f"w2_{i}", name=f"w2_{i}")
                  for i in range(8)]
            w3 = [consts.tile([128, 512], BF16, tag=f"w3_{i}", name=f"w3_{i}")
                  for i in range(8)]
            w4 = [consts.tile([128, 256], BF16, tag=f"w4_{i}", name=f"w4_{i}")
                  for i in range(4)]
            wp = [consts.tile([128, 1], BF16, tag=f"wp{i}", name=f"wp{i}")
                  for i in range(2)]
            b2 = consts.tile([128, 8], F32, tag="b2", name="b2")
            b3 = consts.tile([128, 4], F32, tag="b3", name="b3")
            b4 = consts.tile([128, 2], F32, tag="b4", name="b4")
            bp = consts.tile([1, 1], F32, tag="bp", name="bp")

            deferred = [(bp, bp_d[:]), (b2, b2_d[:]), (b3, b3_d[:]),
                        (b4, b4_d[:]), (w1x, w1x_d[:]),
                        (w1h[0], w1h_d[0]), (w1h[1], w1h_d[1])]
            deferred += [(w2[i], w2_d[i]) for i in range(8)]
            deferred += [(w3[i], w3_d[i]) for i in range(8)]
            deferred += [(w4[i], w4_d[i]) for i in range(4)]
            deferred += [(wp[i], wp_d[i]) for i in range(2)]

            def pump_deferred(nmax):
                for _ in range(nmax):
                    if deferred:
                        t_, ap_ = deferred.pop(0)
                        nc.sync.dma_start(t_[:], ap_)

            oT = consts.tile([1, BL], F32, tag="oT", name="oT")

            mm = nc.tensor.matmul
            h_cur = [None] * NG
            pending = []  # [(t, g, rz, n, d)] 2-deep h-update pipeline

            def flush_tail():
                if not pending:
                    return
                tt, gg, rz_p, n_p, d_p = pending.pop(0)
                h_new = work.tile([128, 1024], BF16, tag=f"h{gg}",
                                  name=f"h{gg}", bufs=2)
                a_ = work.tile([128, 1024], BF16, tag="a", name="a", bufs=2)
                if tt == 0:
                    nc.vector.tensor_mul(a_[:], rz_p[:, 1024:2048], n_p[:])
                    nc.vector.tensor_sub(h_new[:], n_p[:], a_[:])
                else:
                    nc.vector.tensor_mul(a_[:], rz_p[:, 1024:2048], d_p[:])
                    nc.vector.tensor_add(h_new[:], n_p[:], a_[:])
                h_cur[gg] = h_new

            # ---------------- GRU over V=20 vehicle steps ----------------
            for t in range(V):
                xa = xtiles[t] if t in xtiles else xtile(t)
                xtile(min(t + 2, V - 1))  # prefetch
                if t >= 1:
                    pump_deferred(2)
                for g in range(NG):
                    gs = slice(GN * g, GN * (g + 1))

                    pR = psum.tile([128, 1024], F32, tag="pR", name="pR", bufs=1)
                    pZ = psum.tile([128, 1024], F32, tag="pZ", name="pZ", bufs=1)
                    pXN = psum.tile([128, 1024], F32, tag="pXN", name="pXN",
                                    bufs=1)
                    pHN = None
                    if t > 0:
                        pHN = psum.tile([128, 1024], F32, tag="pHN", name="pHN",
                                        bufs=1)

                    rz_dst = [pR[:, 0:512], pR[:, 512:1024],
                              pZ[:, 0:512], pZ[:, 512:1024]]
                    hg = h_cur[g]

                    # r/z: h-side then x-side per chunk (pR chunks first so
                    # the sigmoid drain of the previous group overlaps)
                    for c in range(4):
                        s = 32 * c
                        if t > 0:
                            for k in range(2):
                                mm(rz_dst[c],
                                   whh[k][:, 128 * c : 128 * (c + 1)],
                                   hg[:, 512 * k : 512 * (k + 1)],
                                   start=(k == 0), stop=False)
                        mm(rz_dst[c],
                           wih[s : s + 16, 128 * c : 128 * (c + 1)],
                           xa[s : s + 16, gs],
                           start=(t == 0), stop=True,
                           tile_position=(s, 0))

                    # hn h-side (t>0) — trails so prev group's u-read of pHN
                    # has time to drain
                    if t > 0:
                        for m in range(2):
                            for k in range(2):
                                mm(pHN[:, 512 * m : 512 * (m + 1)],
                                   whh[k][:, 512 + 128 * m : 512 + 128 * (m + 1)],
                                   hg[:, 512 * k : 512 * (k + 1)],
                                   start=(k == 0), stop=(k == 1))

                    # xn x-side last — prev group's w-read of pXN drains late
                    for m in range(2):
                        s = 32 * m
                        mm(pXN[:, 512 * m : 512 * (m + 1)],
                           wih[s : s + 16, 128 * (4 + m) : 128 * (5 + m)],
                           xa[s : s + 16, gs],
                           start=True, stop=True,
                           tile_position=(s, 0))

                    rz = work.tile([128, 2048], BF16, tag="rz", name="rz",
                                   bufs=3)
                    nc.scalar.activation(rz[:, 0:1024], pR[:], Act.Sigmoid)
                    nc.scalar.activation(rz[:, 1024:2048], pZ[:], Act.Sigmoid)

                    # flush the h-update from two groups ago
                    if len(pending) >= 2:
                        flush_tail()

                    u_ = work.tile([128, 1024], BF16, tag="u", name="u", bufs=2)
                    if t == 0:
                        for m in range(2):
                            nc.vector.tensor_scalar_mul(
                                u_[:, 512 * m : 512 * (m + 1)],
                                rz[:, 512 * m : 512 * (m + 1)],
                                bhn[:, m : m + 1])
                    else:
                        for m in range(2):
                            nc.vector.scalar_tensor_tensor(
                                u_[:, 512 * m : 512 * (m + 1)],
                                pHN[:, 512 * m : 512 * (m + 1)],
                                bhn[:, m : m + 1],
                                rz[:, 512 * m : 512 * (m + 1)],
                                op0=Alu.add, op1=Alu.mult)
                    w_ = work.tile([128, 1024], BF16, tag="w", name="w", bufs=2)
                    nc.vector.tensor_add(w_[:], u_[:], pXN[:])
                    n_ = work.tile([128, 1024], BF16, tag="n", name="n", bufs=3)
                    nc.scalar.activation(n_[:], w_[:], Act.Tanh)

                    d_ = None
                    if t > 0:
                        d_ = work.tile([128, 1024], BF16, tag="d", name="d",
                                       bufs=3)
                        nc.gpsimd.tensor_sub(d_[:], h_cur[g][:], n_[:])

                    pending.append((t, g, rz, n_, d_))

            while pending:
                flush_tail()
            pump_deferred(99)

            # ---------------- MLP head ----------------
            GSL = [slice(GN * g, GN * (g + 1)) for g in range(NG)]
            tagp = [("pR", "pZ"), ("pXN", "pHN")]

            def mlp_layer(n_out_chunks, k_tiles, rhs_of, w_of, bias_of,
                          out_tag, out_bufs):
                outs = []
                for m_ in range(n_out_chunks):
                    tg = tagp[m_ % 2]
                    pq0 = psum.tile([128, 1024], F32, tag=tg[0], name="pq0",
                                    bufs=1)
                    pq1 = psum.tile([128, 1024], F32, tag=tg[1], name="pq1",
                                    bufs=1)
                    halves = [pq0[:, 0:512], pq0[:, 512:1024],
                              pq1[:, 0:512], pq1[:, 512:1024]]
                    for ki in range(k_tiles):
                        for g in range(NG):
                            mm(halves[g], w_of(ki, m_), rhs_of(ki, g),
                               start=(ki == 0), stop=(ki == k_tiles - 1))
                    a_t = mlp.tile([128, BL], BF16, tag=out_tag, name=out_tag,
                                   bufs=out_bufs)
                    bias = bias_of(m_)
                    if bias is None:
                        nc.scalar.activation(a_t[:, 0:1024], pq0[:], Act.Relu)
                        nc.scalar.activation(a_t[:, 1024:2048], pq1[:],
                                             Act.Relu)
                    else:
                        nc.scalar.activation(a_t[:, 0:1024], pq0[:], Act.Relu,
                                             bias=bias)
                        nc.scalar.activation(a_t[:, 1024:2048], pq1[:],
                                             Act.Relu, bias=bias)
                    outs.append(a_t)
                return outs

            # layer-1: k0 = x features (+b1 on ones-row), k1/k2 = h chunks
            def l1_rhs(ki, g):
                if ki == 0:
                    return x0[0:16, GSL[g]]
                k = ki - 1
                return h_cur[g][:, 512 * k : 512 * (k + 1)]

            def l1_w(ki, m_):
                if ki == 0:
                    return w1x[:, 128 * m_ : 128 * (m_ + 1)]
                return w1h[ki - 1][:, 128 * m_ : 128 * (m_ + 1)]

            a1 = mlp_layer(8, 3, l1_rhs, l1_w, lambda m_: None, "a1", 8)
            a2 = mlp_layer(8, 8, lambda ki, g: a1[ki][:, GSL[g]],
                           lambda ki, m_: w2[ki][:, 128 * m_ : 128 * (m_ + 1)],
                           lambda m_: b2[:, m_ : m_ + 1], "a2", 8)
            # a3/a4 reuse a1/a2 buffers (generation 2) — dead by then
            a3 = mlp_layer(4, 8, lambda ki, g: a2[ki][:, GSL[g]],
                           lambda ki, m_: w3[ki][:, 128 * m_ : 128 * (m_ + 1)],
                           lambda m_: b3[:, m_ : m_ + 1], "a1", 8)
            a4 = mlp_layer(2, 4, lambda ki, g: a3[ki][:, GSL[g]],
                           lambda ki, m_: w4[ki][:, 128 * m_ : 128 * (m_ + 1)],
                           lambda m_: b4[:, m_ : m_ + 1], "a2", 8)

            po0 = psum.tile([1, 1024], F32, tag="pR", name="po0", bufs=1)
            po1 = psum.tile([1, 1024], F32, tag="pZ", name="po1", bufs=1)
            phalf = [po0[0:1, 0:512], po0[0:1, 512:1024],
                     po1[0:1, 0:512], po1[0:1, 512:1024]]
            for ki in range(2):
                for g in range(NG):
                    mm(phalf[g], wp[ki][:, 0:1], a4[ki][:, GSL[g]],
                       start=(ki == 0), stop=(ki == 1))
            nc.scalar.activation(oT[0:1, 0:1024], po0[0:1, :], Act.Tanh,
                                 bias=bp[0:1, 0:1])
            nc.scalar.activation(oT[0:1, 1024:2048], po1[0:1, :], Act.Tanh,
                                 bias=bp[0:1, 0:1])

            nc.sync.dma_start(out_d[:], oT[:])

    nc.compile()
    return nc


def _get_nc():
    if "nc" not in _NC_CACHE:
        _NC_CACHE["nc"] = _build_nc()
    return _NC_CACHE["nc"]


def _prep_shared(inputs):
    f4 = np.float32
    bf = ml_dtypes.bfloat16

    def g(name):
        return np.asarray(inputs[name], dtype=f4)

    W_ih, W_hh = g("W_ih"), g("W_hh")
    b_ih, b_hh = g("b_ih"), g("b_hh")
    W1, W2, W3, W4, Wp = g("W1"), g("W2"), g("W3"), g("W4"), g("Wp")
    b1, b2, b3, b4, bp = g("b1"), g("b2"), g("b3"), g("b4"), g("bp")

    # wih_all [128, 768]: 6 column blocks; block c sits at partition strip
    # 32*(c%4). blocks 0-3: r/z gate chunks (ones-row bias = b_ih+b_hh);
    # blocks 4-5: xn chunks (ones-row bias = b_ih n-part).
    wih_all = np.zeros((128, 768), dtype=f4)
    bsum = b_ih + b_hh
    for c in range(4):
        s = 32 * c
        wih_all[s : s + 15, 128 * c : 128 * (c + 1)] = W_ih[128 * c : 128 * (c + 1)].T
        wih_all[s + 15, 128 * c : 128 * (c + 1)] = bsum[128 * c : 128 * (c + 1)]
    for m in range(2):
        c, s = 4 + m, 32 * m
        lo = 512 + 128 * m
        wih_all[s : s + 15, 128 * c : 128 * (c + 1)] = W_ih[lo : lo + 128].T
        wih_all[s + 15, 128 * c : 128 * (c + 1)] = b_ih[lo : lo + 128]

    w1x = np.zeros((16, 1024), dtype=f4)
    w1x[0:15] = W1.T[0:15]
    w1x[15] = b1

    shared = {
        "wih_all": wih_all.astype(bf),
        "w_hhT": np.ascontiguousarray(W_hh.T.reshape(2, 128, 768)).astype(bf),
        "w1T_x": w1x.astype(bf),
        "w1T_h": np.ascontiguousarray(W1.T[15:].reshape(2, 128, 1024)).astype(bf),
        "w2T": np.ascontiguousarray(W2.T.reshape(8, 128, 1024)).astype(bf),
        "w3T": np.ascontiguousarray(W3.T.reshape(8, 128, 512)).astype(bf),
        "w4T": np.ascontiguousarray(W4.T.reshape(4, 128, 256)).astype(bf),
        "wpT": np.ascontiguousarray(Wp.T.reshape(2, 128, 1)).astype(bf),
        "bhh_n": np.ascontiguousarray(b_hh[512:768].reshape(2, 128).T),
        "b2": np.ascontiguousarray(b2.reshape(8, 128).T),
        "b3": np.ascontiguousarray(b3.reshape(4, 128).T),
        "b4": np.ascontiguousarray(b4.reshape(2, 128).T),
        "bp": bp.reshape(1, 1).astype(f4),
    }
    return shared


def _prep_xT(state_core):
    """state [BL, 20, 15] f32 -> [20, 128, BL] bf16: features replicated at
    partition strips 0/32/64/96; row 15 of each strip is the all-ones bias
    row; rows 16-31 zero."""
    s = state_core.transpose(1, 2, 0)  # [20, 15, BL]
    xp = np.zeros((V, 4, 32, BL), dtype=np.float32)
    xp[:, :, :F, :] = s[:, None, :, :]
    xp[:, :, 15, :] = 1.0
    return xp.reshape(V, 128, BL).astype(ml_dtypes.bfloat16)


def run(inputs, trace=False):
    nc = _get_nc()
    shared = _prep_shared(inputs)
    state = np.asarray(inputs["state"], dtype=np.float32)
    in_maps = []
    for c in range(NCORES):
        m = dict(shared)
        m["xT"] = _prep_xT(state[BL * c : BL * (c + 1)])
        in_maps.append(m)
    res = run_bass_kernel_spmd(nc, in_maps, list(range(NCORES)), trace=trace)
    out = np.concatenate(
        [np.asarray(res.results[c]["out"]).reshape(BL) for c in range(NCORES)]
    )
    return out.reshape(B, 1).astype(np.float32), res


def kernel(**inputs):
    out, _ = run(inputs, trace=False)
    return out
